# revision 54
# baseline (speedup 1.0000x reference)
"""Trainium2 Bass kernel for per-(b,v)-slice masked attention.

Reference computation (per (b,v) slice, P=S=512, D=512):
    q = X_q @ Wq.T + bq          (softmax scale folded into Wq here)
    k = X_k @ Wkv.T + bkv
    v = X_v @ Wkv.T + bkv
    scores = q @ k.T, diag masked, attn = softmax(scores)
    out = (attn @ v) @ Wo.T + bo

Sharding: 128 (b,v) slices split 16-per-core across 8 cores; weights
replicated. The host pre-transposes q/k activations to d-major layout so
every on-chip matmul contracts over the partition dimension.

Fast path (bq == bkv == 0, which setup_inputs guarantees): fold the
weight products on the host —
    M  = (scale*Wq).T @ Wkv     so  scores.T = Xk @ M.T @ Xq.T
    N0 = Wkv.T @ Wo.T           so  out = (attn @ Xv) @ N0 + (Wo@bkv + bo)
eliminating the k and v projections (4 big matmul groups per slice
instead of 6). Per slice:
    u[d,p]    = (M tiles).T @ XqT
    sT[s,p]   = (XkT tiles).T @ u       (scores transposed)
    eT[s,p]   = exp(sT) * (1 - I)      (diag mask, multiplicative)
    eF[s,p]   = sum_i eT chunks        (gpsimd folds; denominators then
                                        need 4 one-col matmuls, not 16 --
                                        each 1-col matmul pays ~25 ns of
                                        unhidable LDWEIGHTS)
    sums[p]   = (eF tiles).T @ ones    (softmax denominator, [128,4] psum)
    axT[d,p]  = (Xv tiles).T @ eT      (attn @ Xv, transposed, unnormalized)
    out[p,o]  = ((axT tiles).T @ N0) * recip(sums)[p] + bo2

Emission is software-pipelined per iteration as
    u(s) | ax(s-1) | out(s-1)+sums+epilogue | scores(s)+exp
which (a) gives the scalar-serial exp chain and the gpsimd masks/folds a
full iteration of slack before ax/sums consume them, (b) keeps the
scalar queue in [ax evacs, epilogues, exps] order so out PSUM banks
recycle before the data-gated exps, and (c) lets walrus (which list-
schedules the PE stream) find no stalls.  PSUM: scores get a dedicated
4-bank pool (their banks are only reused by scores(s+1), long after the
exps read them); u/ax/out rotate 3 banks; sums hold the 8th.

Startup: the HAM clock gate needs ~3.4+ us of continuous PE busy to
promote 1.2 -> 2.0 -> 2.37 GHz, and an idle gap mid-ramp can leave the
clock parked at 2.0 GHz for the WHOLE kernel (+40 us); 42 dummy matmuls
bridge the ramp plus the startup-DMA latency.  Startup loads are split
across the sync and scalar queues (the only hardware-DGE triggers;
gpsimd dma_start is software-DGE and microseconds late), kk-granular so
slice 0's u phases chase the landing chunks, with slice 1's xq half
prefetched before mask/ones (walrus runs u(1) right behind u(0)).
Drain: the last slice computes sums straight from the eT chunks
(16 interleaved 1-col matmuls) since the eF fold chain would gate the
final epilogues with no other PE work left to hide it.

General path (nonzero bq/bkv): explicit q/k/v projections as above.

Matmul operands are bf16 (PE streams 1 cycle/row with fast weight load);
all PSUM accumulation and softmax normalization stay fp32.  fp8
(DoubleRow, 2x PE throughput) was measured numerically and rejected:
even one matmul group in e4m3 yields rel_err ~0.05 vs the 2e-2 gate.
"""

import numpy as np
import ml_dtypes

import concourse.bacc as bacc
import concourse.mybir as mybir
from concourse.tile import TileContext
from concourse.bass_utils import run_bass_kernel_spmd

B, V, P, D = 4, 32, 512, 512
N_CORES = 8
SLICES = B * V  # 128
SPC = SLICES // N_CORES  # 16 slices per core
KT = D // 128  # 4 contraction tiles
PT = P // 128  # 4 token tiles

BF16 = mybir.dt.bfloat16
F16 = mybir.dt.float16
F32 = mybir.dt.float32
AF = mybir.ActivationFunctionType
ALU = mybir.AluOpType


def _new_nc():
    return bacc.Bacc("TRN2", target_bir_lowering=False, debug=False,
                     num_devices=N_CORES)


def _load_w(nc, cpool, dram, engine=None):
    t = cpool.tile([128, KT, D], BF16, tag=dram.name)
    src = dram.ap().rearrange("(kk p) f -> p kk f", p=128)
    (engine or nc.sync).dma_start(out=t[:], in_=src)
    return t


def _load_x(nc, xpool, dram, s, tag, engine=None):
    t = xpool.tile([128, KT, P], BF16, tag=tag)
    src = dram.ap()[s].rearrange("(kk p) f -> p kk f", p=128)
    (engine or nc.sync).dma_start(out=t[:], in_=src)
    return t


def build_program_fast(zero_bo=False):
    """Zero-bias fast path: 4 matmul groups per slice.

    zero_bo: the output bias is all-zero (true for the graded inputs), so
    the epilogue is a pure rcp-scale on the vector engine (no bias read,
    no bias load on the startup ring).
    """
    nc = _new_nc()

    # All activation layouts are partition-major on the host so every DMA is
    # 4-8 KiB contiguous per partition (big descriptors; the ring-serialized
    # per-call overhead is ~0.6 us, so few large calls beat many small ones).
    # mq0h packs [Mh | xq(0)]: slice 0's critical path is two 512 KiB calls,
    # and group 1 of slice 0 runs half-outer so the PE starts on the first
    # half while the second is still in flight.
    mq0_d = nc.dram_tensor("mq0h", [128, KT, 2, P], BF16, kind="ExternalInput")
    xqk_d = nc.dram_tensor("xqkT", [SPC, 128, 2, KT, P], BF16,
                           kind="ExternalInput")
    xv_d = nc.dram_tensor("xvN", [SPC, 128, PT, D], BF16, kind="ExternalInput")
    n0_d = nc.dram_tensor("N0h", [128, KT, D], BF16, kind="ExternalInput")
    bo2_d = nc.dram_tensor("bo2_bc", [128, D], F32, kind="ExternalInput")
    mask_d = nc.dram_tensor("mask", [128, 128], BF16, kind="ExternalInput")
    ones_d = nc.dram_tensor("ones1", [128, 1], F16, kind="ExternalInput")
    onesb_d = nc.dram_tensor("ones1b", [128, 1], BF16, kind="ExternalInput")
    out_d = nc.dram_tensor("out", [SPC, PT, 128, D], BF16,
                           kind="ExternalOutput")

    with TileContext(nc) as tc:
        with (
            tc.tile_pool(name="consts", bufs=1) as cpool,
            tc.tile_pool(name="xin", bufs=3) as xpool,
            tc.tile_pool(name="proj", bufs=3) as ppool,
            tc.tile_pool(name="attn", bufs=3) as apool,
            tc.tile_pool(name="outp", bufs=3) as opool,
            tc.tile_pool(name="small", bufs=3) as spool,
            # u/ax/out rotate 3 PSUM banks (12 allocs/slice); scores get a
            # dedicated 4-bank pool so the banks the exps read are never
            # reused by the next iteration's early matmuls -- the exp-read ->
            # matmul-write antidependency was a ~0.4 us/slice stall
            tc.tile_pool(name="psum", bufs=3, space="PSUM") as mmpool,
            tc.tile_pool(name="psum_sc", bufs=4, space="PSUM") as scpool,
            tc.tile_pool(name="psum_sums", bufs=1, space="PSUM") as sumpool,
        ):
            # PE warmup: the HAM clock gate needs ~3.4 us of CONTINUOUS busy
            # to promote to the top pstate, and an idle gap mid-ramp can
            # leave the clock stuck at 2.0 GHz for the whole kernel (a 40+ us
            # loss).  42 dummy matmuls bridge the ramp window plus the
            # trigger->land latency of the startup DMAs.  The memset rides
            # gpsimd, whose queue comes up first.
            zq = cpool.tile([128, 256], BF16, tag="zq")
            nc.gpsimd.memset(zq[:], 0.0)
            wps = mmpool.tile([128, 128], F32, tag="mm")
            for _ in range(42):
                nc.tensor.matmul(wps[:], lhsT=zq[:, 128:], rhs=zq[:, :128],
                                 start=True, stop=True)

            # Slice-0-critical loads are spread across four engine queues so
            # the chunks land in parallel; the scalar queue's head is blocked
            # ~1.3 us by the hoisted ACT_TABLE_LOAD, so it gets the last
            # chunk.  Everything else keeps need-order within its queue.
            mq0 = cpool.tile([128, KT, 2, P], BF16, tag="mq0")
            xqk0 = xpool.tile([128, 2, KT, P], BF16, tag="xqk")
            mask_sb = cpool.tile([128, 128], BF16, tag="mask")
            ones_sb = cpool.tile([128, 1], F16, tag="ones1")
            # gpsimd dma_start is software-DGE (descriptor generation in
            # software, microseconds late) -- only sync and scalar trigger
            # the hardware DGE rings
            nc.sync.dma_start(out=mq0[:, 0:1], in_=mq0_d.ap()[:, 0:1])
            nc.sync.dma_start(out=mq0[:, 1:2], in_=mq0_d.ap()[:, 1:2])
            nc.scalar.dma_start(out=mq0[:, 2:3], in_=mq0_d.ap()[:, 2:3])
            nc.scalar.dma_start(out=mq0[:, 3:4], in_=mq0_d.ap()[:, 3:4])
            nc.sync.dma_start(out=xqk0[:, 1, 0:2], in_=xqk_d.ap()[0, :, 1, 0:2])
            nc.sync.dma_start(out=xqk0[:, 1, 2:4],
                              in_=xqk_d.ap()[0, :, 1, 2:4])
            # walrus list-schedules u(1) right behind u(0), so slice 1's xq
            # half must be in flight before the mask/ones triggers -- a 1 MiB
            # xqk(1) dispatched from front_u(0) landed ~6.5 us late
            xqk1 = xpool.tile([128, 2, KT, P], BF16, tag="xqk")
            nc.scalar.dma_start(out=xqk1[:, 0], in_=xqk_d.ap()[1, :, 0])
            nc.scalar.dma_start(out=mask_sb[:], in_=mask_d.ap())
            nc.scalar.dma_start(out=ones_sb[:], in_=ones_d.ap())
            xv0 = xpool.tile([128, PT, D], BF16, tag="xv")
            nc.sync.dma_start(out=xv0[:], in_=xv_d.ap()[0])
            n0_sb = cpool.tile([128, KT, D], BF16, tag="N0h")
            nc.sync.dma_start(out=n0_sb[:], in_=n0_d.ap())
            # bf16 ones column for the last slice's per-chunk eT sums; only
            # needed at the drain, so it rides the sync tail
            onesb_sb = cpool.tile([128, 1], BF16, tag="ones1b")
            nc.sync.dma_start(out=onesb_sb[:], in_=onesb_d.ap())
            state = {"n0_sb": n0_sb}
            if not zero_bo:
                bo2_sb = cpool.tile([128, D], F32, tag="bo2")
                nc.sync.dma_start(out=bo2_sb[:], in_=bo2_d.ap())
                state["bo2_sb"] = bo2_sb

            def m_ap(kk, m):
                return mq0[:, kk, 0, 128 * m : 128 * (m + 1)]

            def front_u(s, xqk):
                """u projection for slice s; issues next-slice loads on the
                scalar ring (self-paced prefetch at ~one-slice lead)."""
                xqk_next = None
                if s == 0:
                    # xq(1) is already in flight; fetch only the xk half
                    xqk_next = xqk1
                    nc.scalar.dma_start(out=xqk1[:, 1], in_=xqk_d.ap()[1, :, 1])
                elif s + 1 < SPC:
                    # halves issued at two points of the iteration (the xk
                    # half from the main loop) to halve the peak SBUF-write
                    # burst behind each prefetch
                    xqk_next = xpool.tile([128, 2, KT, P], BF16, tag="xqk")
                    nc.scalar.dma_start(out=xqk_next[:, 0],
                                        in_=xqk_d.ap()[s + 1, :, 0])
                if s == 0:
                    xv = xv0
                else:
                    xv = xpool.tile([128, PT, D], BF16, tag="xv")
                    nc.scalar.dma_start(out=xv[:], in_=xv_d.ap()[s])

                u = ppool.tile([128, KT, P], BF16, tag="u")  # [d1, p]
                xq0 = mq0[:, :, 1] if s == 0 else xqk[:, 0]

                def evac_u(m, ps):
                    # full-width on vector: the scalar queue's head carries
                    # the previous slice's exps at this point, and the 3-deep
                    # mmpool needs u banks freed promptly
                    nc.vector.tensor_copy(u[:, m, :], ps[:])

                if s == 0:
                    # kk-outer phases: matmuls on already-landed mq0 chunks
                    # run while later chunks are still in flight.  The four
                    # simultaneous PSUM tiles come from the (4-deep) scores
                    # pool; the scores rotation just starts shifted by one.
                    pss = []
                    for _m in range(KT):
                        ps_m = scpool.tile([128, P], F32, tag="sc")
                        pss.append(ps_m)
                    for kk in range(KT):
                        for m in range(KT):
                            nc.tensor.matmul(
                                pss[m][:], lhsT=m_ap(kk, m),
                                rhs=xq0[:, kk, :],
                                start=kk == 0, stop=kk == KT - 1)
                            if kk == KT - 1:
                                evac_u(m, pss[m])
                else:
                    for m in range(KT):
                        ps = mmpool.tile([128, P], F32, tag="mm")
                        for kk in range(KT):
                            nc.tensor.matmul(
                                ps[:], lhsT=m_ap(kk, m), rhs=xq0[:, kk, :],
                                start=kk == 0, stop=kk == KT - 1)
                        evac_u(m, ps)
                return (s, xv, u), xqk_next

            def front_scores(st, xqk):
                """scoresT + exp + diag mask for slice s.  Emitted AFTER
                back(s-1) so the exp/evac of these PSUM banks has a full u
                group (~3.4 us) of slack before back(s) reuses them."""
                s, xv, u = st
                xk = xqk[:, 1]
                eT = apool.tile([128, PT, P], BF16, tag="eT")  # [s, p]
                # partial folds of the eT chunks on gpsimd (after each diag
                # mask): the softmax denominators then need only 4 one-column
                # matmuls on eF instead of 16 on eT -- the ~25 ns LDWEIGHTS
                # per 1-col matmul was a conserved ~0.4 us/slice PE cost that
                # no interleaving could hide
                eF2 = spool.tile([128, 2, P], F16, tag="eF2")
                eF = spool.tile([128, P], F16, tag="eF")
                for i in range(PT):
                    ps = scpool.tile([128, P], F32, tag="sc")
                    for kk in range(KT):
                        nc.tensor.matmul(
                            ps[:], lhsT=xk[:, kk, 128 * i : 128 * (i + 1)],
                            rhs=u[:, kk, :], start=kk == 0, stop=kk == KT - 1)
                    nc.scalar.activation(eT[:, i, :], ps[:], AF.Exp)
                    # diag mask on the (idle) gpsimd engine: the strict-FIFO
                    # vector queue would delay this behind copies/epilogues
                    nc.gpsimd.tensor_mul(
                        eT[:, i, 128 * i : 128 * (i + 1)],
                        eT[:, i, 128 * i : 128 * (i + 1)],
                        mask_sb[:],
                    )
                    if i % 2 and s != SPC - 1:
                        nc.gpsimd.tensor_add(
                            eF2[:, i // 2, :], eT[:, i - 1, :], eT[:, i, :])
                if s != SPC - 1:
                    nc.gpsimd.tensor_add(eF[:], eF2[:, 0, :], eF2[:, 1, :])
                return (s, xv, eT, eF)

            def back_ax(st):
                """attn @ Xv for slice s."""
                s, xv, eT, eF = st
                last = s == SPC - 1
                axT = apool.tile([128, KT, P], BF16, tag="axT")  # [d, p]
                if last:
                    ps_sum = sumpool.tile([128, PT], F32, tag="sums")
                else:
                    ps_sum = None
                for m in range(KT):
                    ps = mmpool.tile([128, P], F32, tag="mm")
                    for i in range(PT):
                        nc.tensor.matmul(
                            ps[:], lhsT=xv[:, i, 128 * m : 128 * (m + 1)],
                            rhs=eT[:, i, :], start=i == 0, stop=i == PT - 1)
                        if last:
                            # drain: the eF fold chain (slow gpsimd adds fed
                            # by the tail exps) would gate the epilogues with
                            # no other PE work left; per-chunk eT sums have
                            # the same readiness as the ax matmuls
                            nc.tensor.matmul(
                                ps_sum[:, m : m + 1],
                                lhsT=eT[:, i, 128 * m : 128 * (m + 1)],
                                rhs=onesb_sb[:], start=i == 0,
                                stop=i == PT - 1)
                    nc.scalar.copy(axT[:, m, 0 : P // 2], ps[:, 0 : P // 2])
                    nc.vector.tensor_copy(axT[:, m, P // 2 :], ps[:, P // 2 :])
                return (s, axT, eF, ps_sum)

            def back_out(bst):
                """Denominators + final projection + epilogue + store.

                The four 1-col sums matmuls live here, a full iteration after
                the gpsimd eF folds complete: walrus schedules PE code in
                data-dependency order, and emitting them any earlier gets
                them hoisted right against eF's readiness (a ~0.6 us stall).
                """
                s, axT, eF, ps_sum = bst
                if ps_sum is None:
                    ps_sum = sumpool.tile([128, PT], F32, tag="sums")
                    for m in range(KT):
                        nc.tensor.matmul(
                            ps_sum[:, m : m + 1],
                            lhsT=eF[:, 128 * m : 128 * (m + 1)],
                            rhs=ones_sb[:], start=True, stop=True)
                rcpT = spool.tile([128, PT], F32, tag="rcpT")
                nc.vector.reciprocal(rcpT[:], ps_sum[:])
                ot = opool.tile([128, PT, D], BF16, tag="ot")
                for j in range(PT):
                    ps = mmpool.tile([128, D], F32, tag="mm")
                    for m in range(KT):
                        nc.tensor.matmul(
                            ps[:], lhsT=axT[:, m, 128 * j : 128 * (j + 1)],
                            rhs=state["n0_sb"][:, m, :],
                            start=m == 0, stop=m == KT - 1)
                    if zero_bo:
                        # epilogue = ps * rcp[p]; odd j ride the scalar
                        # engine (activation Copy with per-partition scale)
                        # so the tail and the vector queue see only half the
                        # chain (gpsimd cannot read PSUM)
                        if j % 2:
                            nc.scalar.mul(ot[:, j, :], ps[:],
                                          rcpT[:, j : j + 1])
                        else:
                            nc.vector.tensor_scalar_mul(
                                ot[:, j, :], ps[:], rcpT[:, j : j + 1])
                    else:
                        nc.vector.scalar_tensor_tensor(
                            ot[:, j, :], ps[:], rcpT[:, j : j + 1],
                            state["bo2_sb"][:], ALU.mult, ALU.add,
                        )
                    # per-j store on the (otherwise idle) sync ring so the
                    # final DMA overlaps the epilogue
                    nc.sync.dma_start(out=out_d.ap()[s, j], in_=ot[:, j, :])

            # Software-pipelined emission per iteration:
            #   u(s) | ax(s-1) | out(s-1)+sums+epilogue | scores(s)+exp
            # scores(s) sits LAST so (a) its exps start one PE slot earlier
            # and finish ~2.5 us before ax(s) needs the masked eT, and (b)
            # the scalar queue runs [ax copies, epilogues, exps] in that
            # order -- epilogues before the data-gated exps, so the out PSUM
            # banks recycle promptly (they feed u(s+1) three allocs later).
            prev = None
            xqk_cur = xqk0
            for s in range(SPC):
                st, xqk_next = front_u(s, xqk_cur)
                if prev is not None:
                    bst = back_ax(prev)
                if s >= 1 and xqk_next is not None:
                    nc.scalar.dma_start(out=xqk_next[:, 1],
                                        in_=xqk_d.ap()[s + 1, :, 1])
                if prev is not None:
                    back_out(bst)
                prev = front_scores(st, xqk_cur)
                xqk_cur = xqk_next
            back_out(back_ax(prev))

    nc.compile()
    return nc


def build_program_general():
    """General path with explicit q/k/v projections (nonzero biases)."""
    nc = _new_nc()

    xq_d = nc.dram_tensor("xqT", [SPC, D, P], BF16, kind="ExternalInput")
    xk_d = nc.dram_tensor("xkT", [SPC, D, P], BF16, kind="ExternalInput")
    xv_d = nc.dram_tensor("xvT", [SPC, D, P], BF16, kind="ExternalInput")
    wq_d = nc.dram_tensor("wqT", [D, D], BF16, kind="ExternalInput")
    wkv_d = nc.dram_tensor("wkvT", [D, D], BF16, kind="ExternalInput")
    wo_d = nc.dram_tensor("woT", [D, D], BF16, kind="ExternalInput")
    bq_d = nc.dram_tensor("bq_col", [128, KT], F32, kind="ExternalInput")
    bkv_d = nc.dram_tensor("bkv_col", [128, KT], F32, kind="ExternalInput")
    bkvb_d = nc.dram_tensor("bkv_bc", [128, D], F32, kind="ExternalInput")
    bob_d = nc.dram_tensor("bo_bc", [128, D], F32, kind="ExternalInput")
    mask_d = nc.dram_tensor("mask", [128, 128], BF16, kind="ExternalInput")
    ones_d = nc.dram_tensor("ones1", [128, 1], BF16, kind="ExternalInput")
    out_d = nc.dram_tensor("out", [SPC, P, D], F32, kind="ExternalOutput")

    with TileContext(nc) as tc:
        with (
            tc.tile_pool(name="consts", bufs=1) as cpool,
            tc.tile_pool(name="xin", bufs=2) as xpool,
            tc.tile_pool(name="proj", bufs=2) as ppool,
            tc.tile_pool(name="attn", bufs=2) as apool,
            tc.tile_pool(name="outp", bufs=3) as opool,
            tc.tile_pool(name="small", bufs=2) as spool,
            tc.tile_pool(name="psum", bufs=6, space="PSUM") as mmpool,
            tc.tile_pool(name="psum_sums", bufs=2, space="PSUM") as sumpool,
        ):
            wq_sb = _load_w(nc, cpool, wq_d)
            wkv_sb = _load_w(nc, cpool, wkv_d)
            wo_sb = _load_w(nc, cpool, wo_d)
            bq_sb = cpool.tile([128, KT], F32, tag="bq")
            nc.sync.dma_start(out=bq_sb[:], in_=bq_d.ap())
            bkv_sb = cpool.tile([128, KT], F32, tag="bkv")
            nc.sync.dma_start(out=bkv_sb[:], in_=bkv_d.ap())
            bkvb_sb = cpool.tile([128, D], F32, tag="bkvb")
            nc.sync.dma_start(out=bkvb_sb[:], in_=bkvb_d.ap())
            bob_sb = cpool.tile([128, D], F32, tag="bob")
            nc.sync.dma_start(out=bob_sb[:], in_=bob_d.ap())
            mask_sb = cpool.tile([128, 128], BF16, tag="mask")
            nc.sync.dma_start(out=mask_sb[:], in_=mask_d.ap())
            ones_sb = cpool.tile([128, 1], BF16, tag="ones1")
            nc.sync.dma_start(out=ones_sb[:], in_=ones_d.ap())

            for s in range(SPC):
                xq = _load_x(nc, xpool, xq_d, s, "xq")
                xk = _load_x(nc, xpool, xk_d, s, "xk")
                xv = _load_x(nc, xpool, xv_d, s, "xv")

                qT = ppool.tile([128, KT, P], BF16, tag="qT")  # [dout, p]
                kTt = ppool.tile([128, KT, P], BF16, tag="kT")  # [dout, s]
                vn = ppool.tile([128, PT, D], BF16, tag="vn")  # [s, dout]
                for m in range(KT):
                    ps = mmpool.tile([128, P], F32, tag="mm")
                    for kk in range(KT):
                        nc.tensor.matmul(
                            ps[:], lhsT=wq_sb[:, kk, 128 * m : 128 * (m + 1)],
                            rhs=xq[:, kk, :], start=kk == 0, stop=kk == KT - 1)
                    nc.scalar.activation(qT[:, m, :], ps[:], AF.Identity,
                                         bias=bq_sb[:, m : m + 1])
                for m in range(KT):
                    ps = mmpool.tile([128, P], F32, tag="mm")
                    for kk in range(KT):
                        nc.tensor.matmul(
                            ps[:], lhsT=wkv_sb[:, kk, 128 * m : 128 * (m + 1)],
                            rhs=xk[:, kk, :], start=kk == 0, stop=kk == KT - 1)
                    nc.scalar.activation(kTt[:, m, :], ps[:], AF.Identity,
                                         bias=bkv_sb[:, m : m + 1])
                for i in range(PT):
                    ps = mmpool.tile([128, D], F32, tag="mm")
                    for kk in range(KT):
                        nc.tensor.matmul(
                            ps[:], lhsT=xv[:, kk, 128 * i : 128 * (i + 1)],
                            rhs=wkv_sb[:, kk, :], start=kk == 0, stop=kk == KT - 1)
                    nc.vector.tensor_add(vn[:, i, :], ps[:], bkvb_sb[:])

                eT = apool.tile([128, PT, P], BF16, tag="eT")  # [s, p]
                for i in range(PT):
                    ps = mmpool.tile([128, P], F32, tag="mm")
                    for kk in range(KT):
                        nc.tensor.matmul(
                            ps[:], lhsT=kTt[:, kk, 128 * i : 128 * (i + 1)],
                            rhs=qT[:, kk, :], start=kk == 0, stop=kk == KT - 1)
                    nc.scalar.activation(eT[:, i, :], ps[:], AF.Exp)
                    # diag mask on the (idle) gpsimd engine: the strict-FIFO
                    # vector queue would delay this behind copies/epilogues,
                    # and eT readiness gates back()'s matmuls
                    nc.gpsimd.tensor_mul(
                        eT[:, i, 128 * i : 128 * (i + 1)],
                        eT[:, i, 128 * i : 128 * (i + 1)],
                        mask_sb[:],
                    )

                ps_sum = sumpool.tile([128, PT], F32, tag="sums")
                for j in range(PT):
                    for i in range(PT):
                        nc.tensor.matmul(
                            ps_sum[:, j : j + 1],
                            lhsT=eT[:, i, 128 * j : 128 * (j + 1)],
                            rhs=ones_sb[:], start=i == 0, stop=i == PT - 1)
                rcpT = spool.tile([128, PT], F32, tag="rcpT")
                nc.vector.reciprocal(rcpT[:], ps_sum[:])

                avT = apool.tile([128, KT, P], BF16, tag="avT")  # [dv, p]
                for m in range(KT):
                    ps = mmpool.tile([128, P], F32, tag="mm")
                    for i in range(PT):
                        nc.tensor.matmul(
                            ps[:], lhsT=vn[:, i, 128 * m : 128 * (m + 1)],
                            rhs=eT[:, i, :], start=i == 0, stop=i == PT - 1)
                    nc.scalar.copy(avT[:, m, :], ps[:])

                ot = opool.tile([128, PT, D], F32, tag="ot")
                for j in range(PT):
                    ps = mmpool.tile([128, D], F32, tag="mm")
                    for m in range(KT):
                        nc.tensor.matmul(
                            ps[:], lhsT=avT[:, m, 128 * j : 128 * (j + 1)],
                            rhs=wo_sb[:, m, :], start=m == 0, stop=m == KT - 1)
                    nc.vector.scalar_tensor_tensor(
                        ot[:, j, :], ps[:], rcpT[:, j : j + 1], bob_sb[:],
                        ALU.mult, ALU.add,
                    )
                nc.sync.dma_start(
                    out=out_d.ap()[s].rearrange("(j p) f -> p j f", p=128),
                    in_=ot[:],
                )

    nc.compile()
    return nc


def _bf16(a):
    return np.ascontiguousarray(a).astype(ml_dtypes.bfloat16)


def _norm_inputs(queries, keys, values, Wq, bq, Wkv, bkv, Wo, bo):
    return (
        np.asarray(queries, np.float32).reshape(SLICES, P, D),
        np.asarray(keys, np.float32).reshape(SLICES, P, D),
        np.asarray(values, np.float32).reshape(SLICES, P, D),
        np.asarray(Wq, np.float32), np.asarray(bq, np.float32),
        np.asarray(Wkv, np.float32), np.asarray(bkv, np.float32),
        np.asarray(Wo, np.float32), np.asarray(bo, np.float32),
    )


def prep_in_maps_fast(queries, keys, values, Wq, bq, Wkv, bkv, Wo, bo):
    queries, keys, values, Wq, bq, Wkv, bkv, Wo, bo = _norm_inputs(
        queries, keys, values, Wq, bq, Wkv, bkv, Wo, bo)

    scale = np.float32(1.0 / np.sqrt(D))
    # scores.T = Xk @ M.T @ Xq.T with M[d2,d1] = (scale*Wq).T @ Wkv
    Mh = _bf16((Wq * scale).T @ Wkv)           # [d2, d1]
    N0h = _bf16(Wkv.T @ Wo.T)                  # [d, dout]
    # [128, KT, D]: partition p, chunk kk <- row kk*128+p
    N0p = np.ascontiguousarray(N0h.reshape(KT, 128, D).transpose(1, 0, 2))
    bo2 = Wo @ bkv + bo
    bo2_bc = np.ascontiguousarray(np.broadcast_to(bo2, (128, D))).astype(np.float32)
    mask = _bf16(1.0 - np.eye(128, dtype=np.float32))

    qT = _bf16(queries.transpose(0, 2, 1))     # [slices, D, P]
    kT = _bf16(keys.transpose(0, 2, 1))
    # [slices, 128, 2, KT, P]: partition-major, 8 KiB contiguous/partition
    qkT = np.stack([qT, kT], axis=1)           # [slices, 2, D, P]
    qkT = np.ascontiguousarray(
        qkT.reshape(SLICES, 2, KT, 128, P).transpose(0, 3, 1, 2, 4))
    # [slices, 128, PT, D]: partition p, chunk i <- row i*128+p
    vN = np.ascontiguousarray(
        _bf16(values).reshape(SLICES, PT, 128, D).transpose(0, 2, 1, 3))
    # [128, KT, 2, P]: per-kk [Mh | xq(slice0 of the core)] pack
    Mp = Mh.reshape(KT, 128, P).transpose(1, 0, 2)  # [128, KT, P]

    in_maps = []
    for c in range(N_CORES):
        sl = slice(c * SPC, (c + 1) * SPC)
        q0 = qT[c * SPC].reshape(KT, 128, P).transpose(1, 0, 2)
        mq0 = np.ascontiguousarray(
            np.stack([Mp, q0], axis=2))        # [128, KT, 2, P]
        in_maps.append({
            "mq0h": mq0, "xqkT": qkT[sl], "xvN": vN[sl],
            "N0h": N0p, "bo2_bc": bo2_bc, "mask": mask,
            "ones1": np.ones((128, 1), np.float16),
            "ones1b": np.ones((128, 1), ml_dtypes.bfloat16),
        })
    return in_maps


def prep_in_maps_general(queries, keys, values, Wq, bq, Wkv, bkv, Wo, bo):
    queries, keys, values, Wq, bq, Wkv, bkv, Wo, bo = _norm_inputs(
        queries, keys, values, Wq, bq, Wkv, bkv, Wo, bo)

    scale = np.float32(1.0 / np.sqrt(D))
    wqT = _bf16((Wq * scale).T)
    wkvT = _bf16(Wkv.T)
    woT = _bf16(Wo.T)
    bq_col = np.ascontiguousarray((bq * scale).reshape(KT, 128).T)
    bkv_col = np.ascontiguousarray(bkv.reshape(KT, 128).T)
    bkv_bc = np.ascontiguousarray(np.broadcast_to(bkv, (128, D))).astype(np.float32)
    bo_bc = np.ascontiguousarray(np.broadcast_to(bo, (128, D))).astype(np.float32)
    mask = _bf16(1.0 - np.eye(128, dtype=np.float32))

    qT = _bf16(queries.transpose(0, 2, 1))
    kT = _bf16(keys.transpose(0, 2, 1))
    vT = _bf16(values.transpose(0, 2, 1))

    in_maps = []
    for c in range(N_CORES):
        sl = slice(c * SPC, (c + 1) * SPC)
        in_maps.append({
            "xqT": qT[sl], "xkT": kT[sl], "xvT": vT[sl],
            "wqT": wqT, "wkvT": wkvT, "woT": woT,
            "bq_col": bq_col, "bkv_col": bkv_col,
            "bkv_bc": bkv_bc, "bo_bc": bo_bc, "mask": mask,
            "ones1": np.ones((128, 1), ml_dtypes.bfloat16),
        })
    return in_maps


_nc_fast = None
_nc_general = None


def kernel(**inputs):
    global _nc_fast, _nc_general
    bq = np.asarray(inputs["bq"], np.float32)
    bkv = np.asarray(inputs["bkv"], np.float32)
    bo = np.asarray(inputs["bo"], np.float32)
    fast = not (np.any(bq) or np.any(bkv))
    if fast:
        if _nc_fast is None:
            _nc_fast = build_program_fast(zero_bo=not np.any(bo))
        nc, in_maps = _nc_fast, prep_in_maps_fast(**inputs)
    else:
        if _nc_general is None:
            _nc_general = build_program_general()
        nc, in_maps = _nc_general, prep_in_maps_general(**inputs)
    res = run_bass_kernel_spmd(nc, in_maps, core_ids=list(range(N_CORES)))
    outs = []
    for c in range(N_CORES):
        o = np.asarray(res.results[c]["out"]).astype(np.float32)
        if fast:  # [SPC, PT, 128, D] j-major == [SPC, P, D] row-major
            o = o.reshape(SPC, P, D)
        outs.append(o)
    return np.concatenate(outs, axis=0).reshape(B, V, P, D)

